# revision 9
# baseline (speedup 1.0000x reference)
"""Fused LSTM-cell kernel for 8x Trainium2 NeuronCores (Bass/Tile).

Strategy: data-parallel over the batch. Each of the 8 cores handles 512
batch rows and computes all gates over the full hidden dim:

    gates[b, g, h] = x[b,:] @ W[g, h, :] + h_prev[b,:] @ V[g, h, :] + bias[g, h]

The two GEMMs are fused into one K=4096 contraction by concatenating
A = [x | h_prev] and stacking Wf = [W^T; V^T] (shared by all cores).
The 8192 fused output columns are reordered into 16 slabs of 512 where a
slab holds all 4 gates for 128 hidden columns — so each PSUM tile can be
combined into h_next/c_next immediately.

Mixed precision: KT16 k-tiles of the contraction run in fp16 (1 k-tile
per 216ns matmul); the last KT8 k-tiles run in fp8-e4m3 with
MatmulPerfMode.DoubleRow, which contracts TWO k-tiles per 216ns matmul
(2x PE throughput). Measured on the real inputs this lands rel_l2
~1.94e-2 on h_next — inside the 2e-2 gate. fp8/fp16 contributions share
one PSUM group by scaling both products to 256x gates (a16*16 * w16*16;
a8*4 * w8*64); the sigmoid/tanh activations absorb the 1/256 via their
scale parameter, so reconciliation costs zero extra ops.

Schedule details:
- Switching the PE perf mode costs a ~620ns pipeline flush, so each slab
  runs one fp16 phase and one fp8 phase across all its m-tiles, and the
  per-slab phase order alternates so half the slab boundaries join
  same-mode phases.
- Slabs 0+1 run as one 8-group interleaved block: at the head the DMA
  rate is still ramping, and 8-way sharing of each weight chunk halves
  the demand (~207 GB/s) so the PE streams almost stall-free from ~10us.
- Inputs stream on two DMA queues (Sync: weights, Scalar: activations /
  bias / c_prev) so the first weight chunks aren't FIFO-serialized
  behind activations.
- Outputs stage into per-slab [128, MT, HB] SBUF tiles and leave as ONE
  c + ONE h DMA per slab (the per-m-tile version serialized 10 ~650ns
  DMA issues into the kernel tail). The last slab's last m-tile instead
  runs a narrow chunked epilogue straight to DRAM to shorten the final
  dependency chain.
"""

import sys
import numpy as np

for _p in ("/opt/trn_rl_repo", "/root/.axon_site/_ro/trn_rl_repo"):
    if _p not in sys.path:
        sys.path.insert(0, _p)

import ml_dtypes

B = 4096
I_DIM = 2048
H_DIM = 2048
G = 4
N_CORES = 8
BS = B // N_CORES              # 512 batch rows per core
MT = BS // 128                 # 4 m-tiles per core
K_TOT = I_DIM + H_DIM          # 4096 fused contraction
KT = K_TOT // 128              # 32 k-tiles
KT8 = 14                       # k-tiles computed in fp8 DoubleRow (pairs!)
KT16 = KT - KT8                # k-tiles computed in fp16
KP8 = KT8 // 2                 # DoubleRow instructions per group
K16 = KT16 * 128
HB = 128                       # hidden columns per slab
S = H_DIM // HB                # 16 slabs
SLAB_N = G * HB                # 512 output columns per slab (PSUM bank)
SA16, SW16 = 16.0, 16.0        # fp16 operand scales (product 256)
SA8, SW8 = 4.0, 64.0           # fp8 operand scales (product 256)
GSCALE = 256.0                 # PSUM holds 256 * gates
N_WARM = 66                    # PE pre-warm matmuls (HAM clock ramp)

_COMPILED = None
TRACE = False          # test harness sets True to capture an NTFF profile
LAST_EXEC_NS = None
LAST_RESULT = None


def _build_program():
    import concourse.mybir as mybir
    import concourse.tile as tile
    from concourse import bacc

    dt = mybir.dt
    DR = mybir.MatmulPerfMode.DoubleRow
    nc = bacc.Bacc("TRN2", target_bir_lowering=False, debug=False,
                   num_devices=N_CORES)

    a16_dram = nc.dram_tensor("a16_t", [MT, 128, K16], dt.float16,
                              kind="ExternalInput").ap()
    a8_dram = nc.dram_tensor("a8_t", [MT, 128, KT8, 128], dt.float8e4,
                             kind="ExternalInput").ap()
    w16_dram = nc.dram_tensor("w16_sl", [S, 128, KT16, SLAB_N], dt.float16,
                              kind="ExternalInput").ap()
    w8_dram = nc.dram_tensor("w8_sl", [S, 128, KT8, SLAB_N], dt.float8e4,
                             kind="ExternalInput").ap()
    bias_dram = nc.dram_tensor("bias_sl", [S, 128, SLAB_N], dt.float32,
                               kind="ExternalInput").ap()
    cprev_dram = nc.dram_tensor("c_prev_s", [BS, H_DIM], dt.float32,
                                kind="ExternalInput").ap()
    # Outputs laid out [p, s, m, hb] so a whole slab leaves as one DMA
    # whose AP order matches the SBUF staging tile; host reassembles.
    h_out = nc.dram_tensor("h_out", [128, S, MT, HB], dt.float32,
                           kind="ExternalOutput").ap()
    c_out = nc.dram_tensor("c_out", [128, S, MT, HB], dt.float32,
                           kind="ExternalOutput").ap()

    SIG = mybir.ActivationFunctionType.Sigmoid
    TANH = mybir.ActivationFunctionType.Tanh
    INV = 1.0 / GSCALE

    with tile.TileContext(nc) as tc:
        with (
            tc.tile_pool(name="apool", bufs=1) as apool,
            tc.tile_pool(name="wpool", bufs=4) as wpool,
            tc.tile_pool(name="w8pool", bufs=3) as w8pool,
            tc.tile_pool(name="bpool", bufs=3) as bpool,
            tc.tile_pool(name="cppool", bufs=10) as cppool,
            tc.tile_pool(name="psum", bufs=8, space="PSUM") as pspool,
            tc.tile_pool(name="gpool", bufs=2) as gpool,
            tc.tile_pool(name="actpool", bufs=2) as actpool,
            tc.tile_pool(name="tpool", bufs=2) as tpool,
            tc.tile_pool(name="opool", bufs=4) as opool,
            tc.tile_pool(name="stpool", bufs=2) as stpool,
        ):
            # Activations resident in SBUF for the whole kernel.
            a16_all = apool.tile([128, MT, K16], dt.float16, tag="a16_all")
            a8_all = apool.tile([128, MT * KT8, 128], dt.float8e4,
                                tag="a8_all")
            # Pre-warm the PE while the first DMAs land: the HAM clock gate
            # holds the PE at 1.2 GHz until it has been busy ~3.4us, so idling
            # through the DMA head would make the first ~30 real matmuls run
            # at half clock. Throwaway matmuls on a zeroed tile flip it to
            # 2.4 GHz before the real work arrives.
            warm = tpool.tile([128, 128], dt.float16, tag="warm")
            nc.any.memset(warm[:], 0.0)
            ps_w = pspool.tile([128, SLAB_N], dt.float32, tag="ps")
            for _ in range(N_WARM):
                nc.tensor.matmul(ps_w[:, 0:128], warm[:], warm[:])

            # ── DMA priming for the joint slab0+1 block ─────────────────
            # Weights on the Sync queue, activations on the Scalar queue;
            # chunks ordered by first consumption so the two FIFOs drain
            # in lock-step with the matmul stream.
            w16_sbs = [wpool.tile([128, KT16, SLAB_N], dt.float16,
                                  tag="w16_sb", name=f"w16_sb_{i}")
                       for i in range(2)]
            w8_sbs = [w8pool.tile([128, KT8, SLAB_N], dt.float8e4,
                                  tag="w8_sb", name=f"w8_sb_{i}")
                      for i in range(2)]
            bias_sbs = [bpool.tile([128, SLAB_N], dt.float32, tag="bias_sb",
                                   name=f"bias_sb_{i}") for i in range(2)]
            awin = [(0, 2), (2, 4), (4, 8), (8, 13), (13, KT16)]
            for (k0, k1) in awin:
                for s in range(2):
                    nc.sync.dma_start(w16_sbs[s][:, k0:k1, :],
                                      w16_dram[s, :, k0:k1, :])
                for m in range(MT):
                    nc.scalar.dma_start(a16_all[:, m, k0 * 128:k1 * 128],
                                        a16_dram[m][:, k0 * 128:k1 * 128])
            for s in range(2):
                nc.sync.dma_start(w8_sbs[s][:], w8_dram[s])
            for m in range(MT):
                nc.scalar.dma_start(a8_all[:, m * KT8:(m + 1) * KT8, :],
                                    a8_dram[m])
            for s in range(2):
                nc.scalar.dma_start(bias_sbs[s][:], bias_dram[s])

            def alloc_groups(slabs):
                cps, pss = {}, {}
                for s in slabs:
                    for m in range(MT):
                        cp = cppool.tile([128, HB], dt.float32, tag="cp_sb")
                        nc.scalar.dma_start(
                            cp[:], cprev_dram[m * 128:(m + 1) * 128,
                                              s * HB:(s + 1) * HB])
                        cps[(s, m)] = cp
                        pss[(s, m)] = pspool.tile([128, SLAB_N], dt.float32,
                                                  tag="ps",
                                                  name=f"ps_{s}_{m}")
                return cps, pss

            def fp16_phase(groups, pss, w16s, interleave, start, stop):
                if interleave:
                    for kt in range(KT16):
                        for (s, m) in groups:
                            nc.tensor.matmul(
                                pss[(s, m)][:],
                                a16_all[:, m, kt * 128:(kt + 1) * 128],
                                w16s[s][:, kt, :],
                                start=(start and kt == 0),
                                stop=(stop and kt == KT16 - 1))
                else:
                    for (s, m) in groups:
                        for kt in range(KT16):
                            nc.tensor.matmul(
                                pss[(s, m)][:],
                                a16_all[:, m, kt * 128:(kt + 1) * 128],
                                w16s[s][:, kt, :],
                                start=(start and kt == 0),
                                stop=(stop and kt == KT16 - 1))

            def fp8_phase(groups, pss, w8s, interleave, start, stop):
                order = ([(kp, sm) for kp in range(KP8) for sm in groups]
                         if interleave else
                         [(kp, sm) for sm in groups for kp in range(KP8)])
                for kp, (s, m) in order:
                    nc.tensor.matmul(
                        pss[(s, m)][:],
                        a8_all[:, (m * KP8 + kp) * 2:
                               (m * KP8 + kp) * 2 + 2, :],
                        w8s[s][:, kp * 2:kp * 2 + 2, :],
                        start=(start and kp == 0),
                        stop=(stop and kp == KP8 - 1),
                        perf_mode=DR)

            def epilogue(s, m, ps, cp, bias_sb, c_st, h_st):
                # PSUM eviction fused with the per-column bias add; PSUM +
                # bias hold 256*gates, the ACT scale undoes it.
                g_sb = gpool.tile([128, SLAB_N], dt.float32, tag="g_sb")
                nc.vector.tensor_add(g_sb[:], ps[:], bias_sb[:])
                acts = actpool.tile([128, SLAB_N], dt.float32, tag="acts")
                nc.scalar.activation(acts[:, 0:3 * HB], g_sb[:, 0:3 * HB],
                                     SIG, scale=INV)
                nc.scalar.activation(acts[:, 3 * HB:4 * HB],
                                     g_sb[:, 3 * HB:4 * HB], TANH, scale=INV)
                t0 = tpool.tile([128, HB], dt.float32, tag="t0")
                nc.vector.tensor_mul(t0[:], acts[:, 0:HB], cp[:])
                t1 = tpool.tile([128, HB], dt.float32, tag="t1")
                nc.vector.tensor_mul(t1[:], acts[:, HB:2 * HB],
                                     acts[:, 3 * HB:4 * HB])
                nc.vector.tensor_add(c_st[:, m, :], t0[:], t1[:])
                th = tpool.tile([128, HB], dt.float32, tag="th")
                nc.scalar.activation(th[:], c_st[:, m, :], TANH)
                nc.vector.tensor_mul(h_st[:, m, :], acts[:, 2 * HB:3 * HB],
                                     th[:])

            def final_epilogue(s, m, ps, cp, bias_sb):
                # Fully exposed after the last matmul: ACT f,i first, then
                # c-tilde, then o (only needed one op later), post-ACT chain
                # in two 64-col chunks straight to DRAM.
                g_sb = gpool.tile([128, SLAB_N], dt.float32, tag="g_sb")
                nc.vector.tensor_add(g_sb[:], ps[:], bias_sb[:])
                acts = actpool.tile([128, SLAB_N], dt.float32, tag="acts")
                nc.scalar.activation(acts[:, 0:2 * HB], g_sb[:, 0:2 * HB],
                                     SIG, scale=INV)
                nc.scalar.activation(acts[:, 3 * HB:4 * HB],
                                     g_sb[:, 3 * HB:4 * HB], TANH, scale=INV)
                nc.scalar.activation(acts[:, 2 * HB:3 * HB],
                                     g_sb[:, 2 * HB:3 * HB], SIG, scale=INV)
                for q in (0, 1):
                    c0, c1 = q * 64, q * 64 + 64
                    t0 = tpool.tile([128, 64], dt.float32, tag="t0")
                    nc.vector.tensor_mul(t0[:], acts[:, c0:c1], cp[:, c0:c1])
                    t1 = tpool.tile([128, 64], dt.float32, tag="t1")
                    nc.vector.tensor_mul(t1[:], acts[:, HB + c0:HB + c1],
                                         acts[:, 3 * HB + c0:3 * HB + c1])
                    c_t = opool.tile([128, 64], dt.float32, tag="c_t")
                    nc.vector.tensor_add(c_t[:], t0[:], t1[:])
                    th = tpool.tile([128, 64], dt.float32, tag="th")
                    nc.scalar.activation(th[:], c_t[:], TANH)
                    h_t = opool.tile([128, 64], dt.float32, tag="h_t")
                    nc.vector.tensor_mul(
                        h_t[:], acts[:, 2 * HB + c0:2 * HB + c1], th[:])
                    nc.scalar.dma_start(c_out[:, s, m, c0:c1], c_t[:])
                    nc.sync.dma_start(h_out[:, s, m, c0:c1], h_t[:])

            # ── Slab-pair blocks: 8 groups (2 slabs x 4 m-tiles) share the
            # 8 PSUM banks, so each pair needs just ONE fp16<->fp8 mode
            # switch, and alternating the pair phase order makes every
            # pair boundary join same-mode phases (8 switches total).
            for p in range(S // 2):
                s0, s1 = 2 * p, 2 * p + 1
                fp8_first = (p % 2 == 1)
                if p == 0:
                    w16s = {0: w16_sbs[0], 1: w16_sbs[1]}
                    w8s = {0: w8_sbs[0], 1: w8_sbs[1]}
                    biases = {0: bias_sbs[0], 1: bias_sbs[1]}
                else:
                    w16s, w8s, biases = {}, {}, {}
                    for s in (s0, s1):
                        w16s[s] = wpool.tile([128, KT16, SLAB_N], dt.float16,
                                             tag="w16_sb", name=f"w16_{s}")
                        w8s[s] = w8pool.tile([128, KT8, SLAB_N], dt.float8e4,
                                             tag="w8_sb", name=f"w8_{s}")
                    if fp8_first:
                        for s in (s0, s1):
                            nc.sync.dma_start(w8s[s][:], w8_dram[s])
                    for s in (s0, s1):
                        for k0, k1 in ((0, 8), (8, 13), (13, KT16)):
                            nc.sync.dma_start(w16s[s][:, k0:k1, :],
                                              w16_dram[s, :, k0:k1, :])
                    if not fp8_first:
                        for s in (s0, s1):
                            nc.sync.dma_start(w8s[s][:], w8_dram[s])
                    for s in (s0, s1):
                        biases[s] = bpool.tile([128, SLAB_N], dt.float32,
                                               tag="bias_sb", name=f"b_{s}")
                        nc.scalar.dma_start(biases[s][:], bias_dram[s])

                cps, pss = alloc_groups([s0, s1])
                groups = [(s, m) for s in (s0, s1) for m in range(MT)]
                # Head pair streams kt-major (8-way chunk sharing while the
                # DMA rate ramps); later pairs run group-major off resident
                # weights so each group's stop lands as early as possible.
                interleave = (p == 0)
                if fp8_first:
                    fp8_phase(groups, pss, w8s, interleave, True, False)
                    fp16_phase(groups, pss, w16s, False, False, True)
                else:
                    fp16_phase(groups, pss, w16s, interleave, True, False)
                    fp8_phase(groups, pss, w8s, False, False, True)

                last_pair = (p == S // 2 - 1)
                for s in (s0, s1):
                    last_slab = (last_pair and s == s1)
                    c_st = stpool.tile([128, MT, HB], dt.float32, tag="c_st",
                                       name=f"c_st_{s}")
                    h_st = stpool.tile([128, MT, HB], dt.float32, tag="h_st",
                                       name=f"h_st_{s}")
                    n_staged = MT - 1 if last_slab else MT
                    for m in range(n_staged):
                        epilogue(s, m, pss[(s, m)], cps[(s, m)], biases[s],
                                 c_st, h_st)
                    nc.sync.dma_start(c_out[:, s, 0:n_staged, :],
                                      c_st[:, 0:n_staged, :])
                    nc.sync.dma_start(h_out[:, s, 0:n_staged, :],
                                      h_st[:, 0:n_staged, :])
                    if last_slab:
                        m = MT - 1
                        final_epilogue(s, m, pss[(s, m)], cps[(s, m)],
                                       biases[s])

    nc.compile()
    return nc


def _prep_inputs(x, h_prev, c_prev, W, bW, V, bV, b):
    e4 = ml_dtypes.float8_e4m3
    x = np.asarray(x, np.float32)
    h_prev = np.asarray(h_prev, np.float32)
    c_prev = np.asarray(c_prev, np.float32)
    W = np.asarray(W, np.float32)
    bW = np.asarray(bW, np.float32)
    V = np.asarray(V, np.float32)
    bV = np.asarray(bV, np.float32)
    b = np.asarray(b, np.float32)

    A = np.concatenate([x, h_prev], axis=1)                      # [B, K]
    A16 = (A[:, :K16] * SA16).astype(np.float16)
    A8 = (A[:, K16:] * SA8).astype(e4)

    # Fused weights, shared by all cores.
    # w16_sl[s, p, kt, g*HB + jj] = WV[g, s*HB + jj, kt*128 + p] * SW16
    WV = np.concatenate([W, V], axis=2)                          # [G, H, K]
    w16_sl = np.ascontiguousarray(
        (WV[:, :, :K16] * SW16).astype(np.float16)
        .reshape(G, S, HB, KT16, 128).transpose(1, 4, 3, 0, 2)
    ).reshape(S, 128, KT16, SLAB_N)
    # w8_sl[s, p, kt8, g*HB + jj] = WV[g, s*HB + jj, K16 + kt8*128 + p] * SW8
    w8_sl = np.ascontiguousarray(
        (WV[:, :, K16:] * SW8).astype(e4)
        .reshape(G, S, HB, KT8, 128).transpose(1, 4, 3, 0, 2)
    ).reshape(S, 128, KT8, SLAB_N)

    bias_full = (bW + bV + b) * GSCALE                           # [G, H]
    bias_row = bias_full.reshape(G, S, HB).transpose(1, 0, 2).reshape(S, SLAB_N)
    bias_sl = np.ascontiguousarray(
        np.broadcast_to(bias_row[:, None, :], (S, 128, SLAB_N))
    ).astype(np.float32)

    in_maps = []
    for c in range(N_CORES):
        r0, r1 = c * BS, (c + 1) * BS
        # a16_t[m, p, kt*128 + j] = A16[r0 + m*128 + j, kt*128 + p]
        a16_t = np.ascontiguousarray(
            A16[r0:r1].reshape(MT, 128, KT16, 128).transpose(0, 3, 2, 1)
        ).reshape(MT, 128, K16)
        # a8_t[m, p, kt8, j] = A8[r0 + m*128 + j, kt8*128 + p]
        a8_t = np.ascontiguousarray(
            A8[r0:r1].reshape(MT, 128, KT8, 128).transpose(0, 3, 2, 1))
        in_maps.append({
            "a16_t": a16_t,
            "a8_t": a8_t,
            "w16_sl": w16_sl,
            "w8_sl": w8_sl,
            "bias_sl": bias_sl,
            "c_prev_s": np.ascontiguousarray(c_prev[r0:r1]),
        })
    return in_maps


def kernel(x, h_prev, c_prev, W, bW, V, bV, b):
    global _COMPILED
    from concourse.bass_utils import run_bass_kernel_spmd

    if _COMPILED is None:
        _COMPILED = _build_program()
    nc = _COMPILED

    in_maps = _prep_inputs(x, h_prev, c_prev, W, bW, V, bV, b)
    res = run_bass_kernel_spmd(nc, in_maps, list(range(N_CORES)), trace=TRACE)
    global LAST_EXEC_NS, LAST_RESULT
    LAST_EXEC_NS = res.exec_time_ns
    LAST_RESULT = res

    # h_out/c_out are [p, s, m, hb]; core rows are m*128+p, cols s*HB+hb.
    def unshard(name):
        parts = []
        for c in range(N_CORES):
            arr = res.results[c][name]                # [128, S, MT, HB]
            parts.append(arr.transpose(2, 0, 1, 3).reshape(BS, H_DIM))
        return np.concatenate(parts, axis=0)

    return (unshard("h_out"), unshard("c_out"))


# revision 10
# speedup vs baseline: 1.0360x; 1.0360x over previous
"""Fused LSTM-cell kernel for 8x Trainium2 NeuronCores (Bass/Tile).

Strategy: data-parallel over the batch. Each of the 8 cores handles 512
batch rows and computes all gates over the full hidden dim:

    gates[b, g, h] = x[b,:] @ W[g, h, :] + h_prev[b,:] @ V[g, h, :] + bias[g, h]

The two GEMMs are fused into one K=4096 contraction by concatenating
A = [x | h_prev] and stacking Wf = [W^T; V^T] (shared by all cores).
The 8192 fused output columns are reordered into 16 slabs of 512 where a
slab holds all 4 gates for 128 hidden columns — so each PSUM tile can be
combined into h_next/c_next immediately.

Mixed precision: KT16 k-tiles of the contraction run in fp16 (1 k-tile
per 216ns matmul); the last KT8 k-tiles run in fp8-e4m3 with
MatmulPerfMode.DoubleRow, which contracts TWO k-tiles per 216ns matmul
(2x PE throughput). Measured on the real inputs this lands rel_l2
~1.94e-2 on h_next — inside the 2e-2 gate. fp8/fp16 contributions share
one PSUM group by scaling both products to 256x gates (a16*16 * w16*16;
a8*4 * w8*64); the sigmoid/tanh activations absorb the 1/256 via their
scale parameter, so reconciliation costs zero extra ops.

Schedule details:
- Switching the PE perf mode costs a ~620ns pipeline flush, so each slab
  runs one fp16 phase and one fp8 phase across all its m-tiles, and the
  per-slab phase order alternates so half the slab boundaries join
  same-mode phases.
- Slabs 0+1 run as one 8-group interleaved block: at the head the DMA
  rate is still ramping, and 8-way sharing of each weight chunk halves
  the demand (~207 GB/s) so the PE streams almost stall-free from ~10us.
- Inputs stream on two DMA queues (Sync: weights, Scalar: activations /
  bias / c_prev) so the first weight chunks aren't FIFO-serialized
  behind activations.
- Outputs stage into per-slab [128, MT, HB] SBUF tiles and leave as ONE
  c + ONE h DMA per slab (the per-m-tile version serialized 10 ~650ns
  DMA issues into the kernel tail). The last slab's last m-tile instead
  runs a narrow chunked epilogue straight to DRAM to shorten the final
  dependency chain.
"""

import sys
import numpy as np

for _p in ("/opt/trn_rl_repo", "/root/.axon_site/_ro/trn_rl_repo"):
    if _p not in sys.path:
        sys.path.insert(0, _p)

import ml_dtypes

B = 4096
I_DIM = 2048
H_DIM = 2048
G = 4
N_CORES = 8
BS = B // N_CORES              # 512 batch rows per core
MT = BS // 128                 # 4 m-tiles per core
K_TOT = I_DIM + H_DIM          # 4096 fused contraction
KT = K_TOT // 128              # 32 k-tiles
KT8 = 14                       # k-tiles computed in fp8 DoubleRow (pairs!)
KT16 = KT - KT8                # k-tiles computed in fp16
KP8 = KT8 // 2                 # DoubleRow instructions per group
K16 = KT16 * 128
HB = 128                       # hidden columns per slab
S = H_DIM // HB                # 16 slabs
SLAB_N = G * HB                # 512 output columns per slab (PSUM bank)
SA16, SW16 = 16.0, 16.0        # fp16 operand scales (product 256)
SA8, SW8 = 4.0, 64.0           # fp8 operand scales (product 256)
GSCALE = 256.0                 # PSUM holds 256 * gates
N_WARM = 66                    # PE pre-warm matmuls (HAM clock ramp)

_COMPILED = None
TRACE = False          # test harness sets True to capture an NTFF profile
LAST_EXEC_NS = None
LAST_RESULT = None


def _build_program():
    import concourse.mybir as mybir
    import concourse.tile as tile
    from concourse import bacc

    dt = mybir.dt
    DR = mybir.MatmulPerfMode.DoubleRow
    nc = bacc.Bacc("TRN2", target_bir_lowering=False, debug=False,
                   num_devices=N_CORES)

    a16_dram = nc.dram_tensor("a16_t", [MT, 128, K16], dt.float16,
                              kind="ExternalInput").ap()
    a8_dram = nc.dram_tensor("a8_t", [MT, 128, KT8, 128], dt.float8e4,
                             kind="ExternalInput").ap()
    w16_dram = nc.dram_tensor("w16_sl", [S, 128, KT16, SLAB_N], dt.float16,
                              kind="ExternalInput").ap()
    w8_dram = nc.dram_tensor("w8_sl", [S, 128, KT8, SLAB_N], dt.float8e4,
                             kind="ExternalInput").ap()
    bias_dram = nc.dram_tensor("bias_sl", [S, 128, SLAB_N], dt.float32,
                               kind="ExternalInput").ap()
    cprev_dram = nc.dram_tensor("c_prev_s", [BS, H_DIM], dt.float32,
                                kind="ExternalInput").ap()
    # Outputs laid out [p, s, m, hb] so a whole slab leaves as one DMA
    # whose AP order matches the SBUF staging tile; host reassembles.
    h_out = nc.dram_tensor("h_out", [128, S, MT, HB], dt.float32,
                           kind="ExternalOutput").ap()
    c_out = nc.dram_tensor("c_out", [128, S, MT, HB], dt.float32,
                           kind="ExternalOutput").ap()

    SIG = mybir.ActivationFunctionType.Sigmoid
    TANH = mybir.ActivationFunctionType.Tanh
    INV = 1.0 / GSCALE

    with tile.TileContext(nc) as tc:
        with (
            tc.tile_pool(name="apool", bufs=1) as apool,
            tc.tile_pool(name="wpool", bufs=4) as wpool,
            tc.tile_pool(name="w8pool", bufs=4) as w8pool,
            tc.tile_pool(name="bpool", bufs=4) as bpool,
            tc.tile_pool(name="cppool", bufs=16) as cppool,
            tc.tile_pool(name="psum", bufs=8, space="PSUM") as pspool,
            tc.tile_pool(name="gpool", bufs=3) as gpool,
            tc.tile_pool(name="actpool", bufs=3) as actpool,
            tc.tile_pool(name="tpool", bufs=2) as tpool,
            tc.tile_pool(name="opool", bufs=4) as opool,
            tc.tile_pool(name="stpool", bufs=3) as stpool,
        ):
            # Activations resident in SBUF for the whole kernel.
            a16_all = apool.tile([128, MT, K16], dt.float16, tag="a16_all")
            a8_all = apool.tile([128, MT * KT8, 128], dt.float8e4,
                                tag="a8_all")
            # Pre-warm the PE while the first DMAs land: the HAM clock gate
            # holds the PE at 1.2 GHz until it has been busy ~3.4us, so idling
            # through the DMA head would make the first ~30 real matmuls run
            # at half clock. Throwaway matmuls on a zeroed tile flip it to
            # 2.4 GHz before the real work arrives.
            warm = tpool.tile([128, 128], dt.float16, tag="warm")
            nc.any.memset(warm[:], 0.0)
            ps_w = pspool.tile([128, SLAB_N], dt.float32, tag="ps")
            for _ in range(N_WARM):
                nc.tensor.matmul(ps_w[:, 0:128], warm[:], warm[:])

            # ── DMA priming for the joint slab0+1 block ─────────────────
            # Weights on the Sync queue, activations on the Scalar queue;
            # chunks ordered by first consumption so the two FIFOs drain
            # in lock-step with the matmul stream.
            w16_sbs = [wpool.tile([128, KT16, SLAB_N], dt.float16,
                                  tag="w16_sb", name=f"w16_sb_{i}")
                       for i in range(2)]
            w8_sbs = [w8pool.tile([128, KT8, SLAB_N], dt.float8e4,
                                  tag="w8_sb", name=f"w8_sb_{i}")
                      for i in range(2)]
            bias_sbs = [bpool.tile([128, SLAB_N], dt.float32, tag="bias_sb",
                                   name=f"bias_sb_{i}") for i in range(2)]
            awin = [(0, 2), (2, 4), (4, 8), (8, 13), (13, KT16)]
            for (k0, k1) in awin:
                for s in range(2):
                    nc.sync.dma_start(w16_sbs[s][:, k0:k1, :],
                                      w16_dram[s, :, k0:k1, :])
                for m in range(MT):
                    nc.scalar.dma_start(a16_all[:, m, k0 * 128:k1 * 128],
                                        a16_dram[m][:, k0 * 128:k1 * 128])
            for s in range(2):
                nc.sync.dma_start(w8_sbs[s][:], w8_dram[s])
            for m in range(MT):
                nc.scalar.dma_start(a8_all[:, m * KT8:(m + 1) * KT8, :],
                                    a8_dram[m])
            for s in range(2):
                nc.scalar.dma_start(bias_sbs[s][:], bias_dram[s])

            def alloc_groups(slabs):
                cps, pss = {}, {}
                for s in slabs:
                    for m in range(MT):
                        cp = cppool.tile([128, HB], dt.float32, tag="cp_sb")
                        nc.scalar.dma_start(
                            cp[:], cprev_dram[m * 128:(m + 1) * 128,
                                              s * HB:(s + 1) * HB])
                        cps[(s, m)] = cp
                        pss[(s, m)] = pspool.tile([128, SLAB_N], dt.float32,
                                                  tag="ps",
                                                  name=f"ps_{s}_{m}")
                return cps, pss

            def fp16_phase(groups, pss, w16s, interleave, start, stop):
                if interleave:
                    for kt in range(KT16):
                        for (s, m) in groups:
                            nc.tensor.matmul(
                                pss[(s, m)][:],
                                a16_all[:, m, kt * 128:(kt + 1) * 128],
                                w16s[s][:, kt, :],
                                start=(start and kt == 0),
                                stop=(stop and kt == KT16 - 1))
                else:
                    for (s, m) in groups:
                        for kt in range(KT16):
                            nc.tensor.matmul(
                                pss[(s, m)][:],
                                a16_all[:, m, kt * 128:(kt + 1) * 128],
                                w16s[s][:, kt, :],
                                start=(start and kt == 0),
                                stop=(stop and kt == KT16 - 1))

            def fp8_phase(groups, pss, w8s, interleave, start, stop):
                order = ([(kp, sm) for kp in range(KP8) for sm in groups]
                         if interleave else
                         [(kp, sm) for sm in groups for kp in range(KP8)])
                for kp, (s, m) in order:
                    nc.tensor.matmul(
                        pss[(s, m)][:],
                        a8_all[:, (m * KP8 + kp) * 2:
                               (m * KP8 + kp) * 2 + 2, :],
                        w8s[s][:, kp * 2:kp * 2 + 2, :],
                        start=(start and kp == 0),
                        stop=(stop and kp == KP8 - 1),
                        perf_mode=DR)

            def epilogue(s, m, ps, cp, bias_sb, c_st, h_st):
                # PSUM eviction fused with the per-column bias add; PSUM +
                # bias hold 256*gates, the ACT scale undoes it.
                g_sb = gpool.tile([128, SLAB_N], dt.float32, tag="g_sb")
                nc.vector.tensor_add(g_sb[:], ps[:], bias_sb[:])
                acts = actpool.tile([128, SLAB_N], dt.float32, tag="acts")
                nc.scalar.activation(acts[:, 0:3 * HB], g_sb[:, 0:3 * HB],
                                     SIG, scale=INV)
                nc.scalar.activation(acts[:, 3 * HB:4 * HB],
                                     g_sb[:, 3 * HB:4 * HB], TANH, scale=INV)
                t0 = tpool.tile([128, HB], dt.float32, tag="t0")
                nc.vector.tensor_mul(t0[:], acts[:, 0:HB], cp[:])
                t1 = tpool.tile([128, HB], dt.float32, tag="t1")
                nc.vector.tensor_mul(t1[:], acts[:, HB:2 * HB],
                                     acts[:, 3 * HB:4 * HB])
                nc.vector.tensor_add(c_st[:, m, :], t0[:], t1[:])
                th = tpool.tile([128, HB], dt.float32, tag="th")
                nc.scalar.activation(th[:], c_st[:, m, :], TANH)
                nc.vector.tensor_mul(h_st[:, m, :], acts[:, 2 * HB:3 * HB],
                                     th[:])

            def final_epilogue(s, m, ps, cp, bias_sb):
                # Fully exposed after the last matmul: ACT f,i first, then
                # c-tilde, then o (only needed one op later), post-ACT chain
                # in two 64-col chunks straight to DRAM.
                g_sb = gpool.tile([128, SLAB_N], dt.float32, tag="g_sb")
                nc.vector.tensor_add(g_sb[:], ps[:], bias_sb[:])
                acts = actpool.tile([128, SLAB_N], dt.float32, tag="acts")
                nc.scalar.activation(acts[:, 0:2 * HB], g_sb[:, 0:2 * HB],
                                     SIG, scale=INV)
                nc.scalar.activation(acts[:, 3 * HB:4 * HB],
                                     g_sb[:, 3 * HB:4 * HB], TANH, scale=INV)
                nc.scalar.activation(acts[:, 2 * HB:3 * HB],
                                     g_sb[:, 2 * HB:3 * HB], SIG, scale=INV)
                for q in (0, 1):
                    c0, c1 = q * 64, q * 64 + 64
                    t0 = tpool.tile([128, 64], dt.float32, tag="t0")
                    nc.vector.tensor_mul(t0[:], acts[:, c0:c1], cp[:, c0:c1])
                    t1 = tpool.tile([128, 64], dt.float32, tag="t1")
                    nc.vector.tensor_mul(t1[:], acts[:, HB + c0:HB + c1],
                                         acts[:, 3 * HB + c0:3 * HB + c1])
                    c_t = opool.tile([128, 64], dt.float32, tag="c_t")
                    nc.vector.tensor_add(c_t[:], t0[:], t1[:])
                    th = tpool.tile([128, 64], dt.float32, tag="th")
                    nc.scalar.activation(th[:], c_t[:], TANH)
                    h_t = opool.tile([128, 64], dt.float32, tag="h_t")
                    nc.vector.tensor_mul(
                        h_t[:], acts[:, 2 * HB + c0:2 * HB + c1], th[:])
                    nc.scalar.dma_start(c_out[:, s, m, c0:c1], c_t[:])
                    nc.sync.dma_start(h_out[:, s, m, c0:c1], h_t[:])

            # ── Slab-pair blocks: 8 groups (2 slabs x 4 m-tiles) share the
            # 8 PSUM banks, so each pair needs just ONE fp16<->fp8 mode
            # switch, and alternating the pair phase order makes every
            # pair boundary join same-mode phases (8 switches total).
            for p in range(S // 2):
                s0, s1 = 2 * p, 2 * p + 1
                fp8_first = (p % 2 == 1)
                if p == 0:
                    w16s = {0: w16_sbs[0], 1: w16_sbs[1]}
                    w8s = {0: w8_sbs[0], 1: w8_sbs[1]}
                    biases = {0: bias_sbs[0], 1: bias_sbs[1]}
                else:
                    w16s, w8s, biases = {}, {}, {}
                    for s in (s0, s1):
                        w16s[s] = wpool.tile([128, KT16, SLAB_N], dt.float16,
                                             tag="w16_sb", name=f"w16_{s}")
                        w8s[s] = w8pool.tile([128, KT8, SLAB_N], dt.float8e4,
                                             tag="w8_sb", name=f"w8_{s}")
                    if fp8_first:
                        for s in (s0, s1):
                            nc.sync.dma_start(w8s[s][:], w8_dram[s])
                    for s in (s0, s1):
                        for k0, k1 in ((0, 8), (8, 13), (13, KT16)):
                            nc.sync.dma_start(w16s[s][:, k0:k1, :],
                                              w16_dram[s, :, k0:k1, :])
                    if not fp8_first:
                        for s in (s0, s1):
                            nc.sync.dma_start(w8s[s][:], w8_dram[s])
                    for s in (s0, s1):
                        biases[s] = bpool.tile([128, SLAB_N], dt.float32,
                                               tag="bias_sb", name=f"b_{s}")
                        nc.scalar.dma_start(biases[s][:], bias_dram[s])

                cps, pss = alloc_groups([s0, s1])
                groups = [(s, m) for s in (s0, s1) for m in range(MT)]
                # Head pair streams kt-major (8-way chunk sharing while the
                # DMA rate ramps); later pairs run group-major off resident
                # weights so each group's stop lands as early as possible.
                interleave = (p == 0)
                if fp8_first:
                    fp8_phase(groups, pss, w8s, interleave, True, False)
                    fp16_phase(groups, pss, w16s, False, False, True)
                else:
                    fp16_phase(groups, pss, w16s, interleave, True, False)
                    fp8_phase(groups, pss, w8s, False, False, True)

                last_pair = (p == S // 2 - 1)
                for s in (s0, s1):
                    last_slab = (last_pair and s == s1)
                    c_st = stpool.tile([128, MT, HB], dt.float32, tag="c_st",
                                       name=f"c_st_{s}")
                    h_st = stpool.tile([128, MT, HB], dt.float32, tag="h_st",
                                       name=f"h_st_{s}")
                    n_staged = MT - 1 if last_slab else MT
                    for m in range(n_staged):
                        epilogue(s, m, pss[(s, m)], cps[(s, m)], biases[s],
                                 c_st, h_st)
                    nc.sync.dma_start(c_out[:, s, 0:n_staged, :],
                                      c_st[:, 0:n_staged, :])
                    nc.sync.dma_start(h_out[:, s, 0:n_staged, :],
                                      h_st[:, 0:n_staged, :])
                    if last_slab:
                        m = MT - 1
                        final_epilogue(s, m, pss[(s, m)], cps[(s, m)],
                                       biases[s])

    nc.compile()
    return nc


def _prep_inputs(x, h_prev, c_prev, W, bW, V, bV, b):
    e4 = ml_dtypes.float8_e4m3
    x = np.asarray(x, np.float32)
    h_prev = np.asarray(h_prev, np.float32)
    c_prev = np.asarray(c_prev, np.float32)
    W = np.asarray(W, np.float32)
    bW = np.asarray(bW, np.float32)
    V = np.asarray(V, np.float32)
    bV = np.asarray(bV, np.float32)
    b = np.asarray(b, np.float32)

    A = np.concatenate([x, h_prev], axis=1)                      # [B, K]
    A16 = (A[:, :K16] * SA16).astype(np.float16)
    A8 = (A[:, K16:] * SA8).astype(e4)

    # Fused weights, shared by all cores.
    # w16_sl[s, p, kt, g*HB + jj] = WV[g, s*HB + jj, kt*128 + p] * SW16
    WV = np.concatenate([W, V], axis=2)                          # [G, H, K]
    w16_sl = np.ascontiguousarray(
        (WV[:, :, :K16] * SW16).astype(np.float16)
        .reshape(G, S, HB, KT16, 128).transpose(1, 4, 3, 0, 2)
    ).reshape(S, 128, KT16, SLAB_N)
    # w8_sl[s, p, kt8, g*HB + jj] = WV[g, s*HB + jj, K16 + kt8*128 + p] * SW8
    w8_sl = np.ascontiguousarray(
        (WV[:, :, K16:] * SW8).astype(e4)
        .reshape(G, S, HB, KT8, 128).transpose(1, 4, 3, 0, 2)
    ).reshape(S, 128, KT8, SLAB_N)

    bias_full = (bW + bV + b) * GSCALE                           # [G, H]
    bias_row = bias_full.reshape(G, S, HB).transpose(1, 0, 2).reshape(S, SLAB_N)
    bias_sl = np.ascontiguousarray(
        np.broadcast_to(bias_row[:, None, :], (S, 128, SLAB_N))
    ).astype(np.float32)

    in_maps = []
    for c in range(N_CORES):
        r0, r1 = c * BS, (c + 1) * BS
        # a16_t[m, p, kt*128 + j] = A16[r0 + m*128 + j, kt*128 + p]
        a16_t = np.ascontiguousarray(
            A16[r0:r1].reshape(MT, 128, KT16, 128).transpose(0, 3, 2, 1)
        ).reshape(MT, 128, K16)
        # a8_t[m, p, kt8, j] = A8[r0 + m*128 + j, kt8*128 + p]
        a8_t = np.ascontiguousarray(
            A8[r0:r1].reshape(MT, 128, KT8, 128).transpose(0, 3, 2, 1))
        in_maps.append({
            "a16_t": a16_t,
            "a8_t": a8_t,
            "w16_sl": w16_sl,
            "w8_sl": w8_sl,
            "bias_sl": bias_sl,
            "c_prev_s": np.ascontiguousarray(c_prev[r0:r1]),
        })
    return in_maps


def kernel(x, h_prev, c_prev, W, bW, V, bV, b):
    global _COMPILED
    from concourse.bass_utils import run_bass_kernel_spmd

    if _COMPILED is None:
        _COMPILED = _build_program()
    nc = _COMPILED

    in_maps = _prep_inputs(x, h_prev, c_prev, W, bW, V, bV, b)
    res = run_bass_kernel_spmd(nc, in_maps, list(range(N_CORES)), trace=TRACE)
    global LAST_EXEC_NS, LAST_RESULT
    LAST_EXEC_NS = res.exec_time_ns
    LAST_RESULT = res

    # h_out/c_out are [p, s, m, hb]; core rows are m*128+p, cols s*HB+hb.
    def unshard(name):
        parts = []
        for c in range(N_CORES):
            arr = res.results[c][name]                # [128, S, MT, HB]
            parts.append(arr.transpose(2, 0, 1, 3).reshape(BS, H_DIM))
        return np.concatenate(parts, axis=0)

    return (unshard("h_out"), unshard("c_out"))
